# revision 18
# baseline (speedup 1.0000x reference)
"""Trainium2 Bass kernel for nn_Att_H (rank-1 attention MLP), 8-core data parallel.

Math (per sample b):
  h = silu(x @ W_in.T + b_in); Q,K,V = silu(h @ A*.T + B*)
  scores[i,j] = Q[i]*K[j]; attn = softmax_j; ctx = silu(attn @ V)
  y = silu(ctx @ W_out.T + b_out); out = quad-form tail on y.

Rank-1 scores => ctx_i = F(Q_i) where F(q) = silu(num(q)/den(q)),
num(q) = sum_j V_j e^{q ktil_j}, den(q) = sum_j e^{q ktil_j}
(ktil = K - Kmax <= 0). F is evaluated exactly at NP=48 grid nodes and
interpolated piecewise-linearly at the 512 Q_i via a relu-basis matmul:
  F(q) = F(t_0) + sum_p w_p relu(q - t_p),  w = 2nd differences of slopes.

Layout tricks vs the v0 kernel (137us):
- NP=96 -> 48 with a retuned two-piece grid (offline sim err 6.3e-3 vs
  gate 2e-2).
- Phase A pair-packed: one S matmul per sample pair (stationary
  ktil8 [8,128] = nearly free LDWEIGHTS), one Exp per 2 pairs, and
  nd matmuls seg-packed over 2 pairs ([128,8] stationary, garbage
  cross-cells discarded at drain).
- 2-pair knot packing: both pairs of a 4-sample group share one
  [94, 2, 512] basis tile (47 knots each half), halving elementwise
  basis work; built alternately on ACT (relu bias trick) and DVE
  (max-sub), sourced from 8 bulk q-broadcast DMAs (3MB total).
- nd drains: DVE-staged, then 16 strided DMAs into [64, 2, 48].
- Node math split in halves so phase B can start before phase A ends.
- Big fp16 warm-up matmuls + early param DMAs attack the PE p-state.
"""

import sys
import numpy as np

for _p in ("/opt/trn_rl_repo", "/opt/trn_rl_repo/concourse"):
    if _p not in sys.path:
        sys.path.append(_p)

B_GLOBAL = 512
N_CORES = 8
B = B_GLOBAL // N_CORES  # 64 samples per core
IN = 128
H = 512
O = 25
NSEG = H // 128  # 4

# PWL grid: 48 nodes, two-piece linear (dense low where Q clusters)
GRID_LO, GRID_MID, GRID_HI = -0.36, 6.0, 44.0
N_LO = 36
NP = 48
NK = NP - 1   # 47 knots
NG = 16       # groups of 4 samples (2 pairs)
BOT = 64      # base partition of the second pair's knot block (HW requires
              # matmul/transpose base partitions in {0, 32, 64})
NROW = BOT + NK  # 111


def _grid():
    t = np.concatenate([
        np.linspace(GRID_LO, GRID_MID, N_LO, endpoint=False),
        np.linspace(GRID_MID, GRID_HI, NP - N_LO),
    ]).astype(np.float16).astype(np.float32)
    return t


_cache = {}


def _build_nc():
    from contextlib import ExitStack

    import concourse.bass as bass
    import concourse.tile as tile
    from concourse import bacc, mybir

    f32 = mybir.dt.float32
    bf16 = mybir.dt.bfloat16
    fp16 = mybir.dt.float16
    EXP = mybir.ActivationFunctionType.Exp
    SILU = mybir.ActivationFunctionType.Silu
    RELU = mybir.ActivationFunctionType.Relu
    AX = mybir.AxisListType.X
    MAX = mybir.AluOpType.max
    SUB = mybir.AluOpType.subtract

    nc = bacc.Bacc()
    x_d = nc.declare_dram_parameter("xT", [IN, B], fp16, False)
    w_inT_d = nc.declare_dram_parameter("w_inT", [IN, H], fp16, False)
    aT_d = [nc.declare_dram_parameter(f"a{m}T", [H, H], fp16, False) for m in "qkv"]
    b_in_d = nc.declare_dram_parameter("b_in_bc", [B, H], fp16, False)
    bb_d = [nc.declare_dram_parameter(f"b{m}_bc", [B, H], fp16, False) for m in "qkv"]
    w_outT_d = nc.declare_dram_parameter("w_outT", [H, O], fp16, False)
    b_out_d = nc.declare_dram_parameter("b_out_bc", [B, O], f32, False)
    eye_d = nc.declare_dram_parameter("eye64", [B, B], f32, False)
    qhat2_d = nc.declare_dram_parameter("qhat2", [8, 2 * NSEG * NP], fp16, False)
    tcol_d = nc.declare_dram_parameter("tcol111", [NROW, 1], f32, False)
    ntcol_d = nc.declare_dram_parameter("ntcol111", [NROW, 1], f32, False)
    invdt_d = nc.declare_dram_parameter("invdt_bc", [B, NK], f32, False)
    out_d = nc.declare_dram_parameter("out", [B, 1], f32, True)
    q_dram = nc.dram_tensor("q_scratch", [B, H], fp16)
    ktil_dram = nc.dram_tensor("ktil_scratch", [B, H], fp16)

    with tile.TileContext(nc) as tc, ExitStack() as ctx:
        const_pool = ctx.enter_context(tc.tile_pool(name="const", bufs=1))
        big_pool = ctx.enter_context(tc.tile_pool(name="big", bufs=1))
        work_pool = ctx.enter_context(tc.tile_pool(name="work", bufs=2))
        stage_pool = ctx.enter_context(tc.tile_pool(name="stg", bufs=2))
        e_pool = ctx.enter_context(tc.tile_pool(name="et", bufs=3))
        bas_pool = ctx.enter_context(tc.tile_pool(name="bas", bufs=3))

        # ---- param loads: early-needed first on sync, rest on gpsimd ----
        xT_sb = const_pool.tile([IN, B], fp16)
        nc.sync.dma_start(xT_sb[:], x_d[:])
        w_inT_sb = const_pool.tile([IN, H], fp16)
        nc.sync.dma_start(w_inT_sb[:], w_inT_d[:])
        b_in_sb = const_pool.tile([B, H], fp16)
        nc.sync.dma_start(b_in_sb[:], b_in_d[:])
        eye_sb = const_pool.tile([B, B], f32)
        nc.sync.dma_start(eye_sb[:], eye_d[:])
        aT_sb = [None, None, None]
        for mi in (1, 0, 2):  # K first: it gates phase A
            t = big_pool.tile([128, NSEG, H], fp16, tag=f"aT{mi}")
            src_r = aT_d[mi][:].rearrange("(s p) i -> p s i", p=128)
            nc.sync.dma_start(t[:, 0:2, :], src_r[:, 0:2, :])
            nc.gpsimd.dma_start(t[:, 2:4, :], src_r[:, 2:4, :])
            aT_sb[mi] = t
        bb_sb = []
        for mi, d in enumerate(bb_d):
            t = const_pool.tile([B, H], fp16, tag=f"bb{mi}")
            nc.gpsimd.dma_start(t[:], d[:])
            bb_sb.append(t)
        qhat2_sb = const_pool.tile([8, 2 * NSEG * NP], fp16)
        nc.gpsimd.dma_start(qhat2_sb[:], qhat2_d[:])
        tcol_sb = const_pool.tile([NROW, 1], f32)
        nc.gpsimd.dma_start(tcol_sb[:], tcol_d[:])
        ntcol_sb = const_pool.tile([NROW, 1], f32)
        nc.gpsimd.dma_start(ntcol_sb[:], ntcol_d[:])
        invdt_sb = const_pool.tile([B, NK], f32)
        nc.gpsimd.dma_start(invdt_sb[:], invdt_d[:])
        w_outT_sb = const_pool.tile([128, NSEG, O], fp16)
        nc.gpsimd.dma_start(w_outT_sb[:], w_outT_d[:].rearrange("(s p) o -> p s o", p=128))
        b_out_sb = const_pool.tile([B, O], f32)
        nc.gpsimd.dma_start(b_out_sb[:], b_out_d[:])

        # big SBUF tensors (rows 47-63 of the knot blocks are dead filler:
        # tcol there is 1e4 so the basis is exactly 0, zbig rows are 0)
        qb_all = big_pool.tile([NROW, NG, 2, H], fp16, tag="qball")
        nc.gpsimd.memset(qb_all[:], 0.0)
        zbig = const_pool.tile([NROW, B * 32 + 64], fp16, tag="zbig")
        nc.vector.memset(zbig[:], 0.0)
        warm_sb = const_pool.tile([128, H], fp16, tag="warm")
        nc.vector.memset(warm_sb[:], 0.0)

        def transpose_to(pool, src_ap, dst_ap):
            """[p0<=64, f<=128] SBUF -> [f, p0] SBUF via PE transpose."""
            p0 = src_ap.shape[0]
            f = src_ap.shape[-1]
            pt = pool.tile([128, B], f32, tag="tp")
            nc.tensor.transpose(pt[0:f, 0:p0], src_ap, eye_sb[0:p0, 0:p0])
            nc.vector.tensor_copy(dst_ap, pt[0:f, 0:p0])

        # =================== phase 0 ===================
        with tc.tile_pool(name="ps0", bufs=2, space="PSUM") as psum_mm:
            # PE warm-up: fat fp16 MMs to push the p-state up while params load
            for wi in range(14):
                wt_ps = psum_mm.tile([128, H], f32, tag="warm", bufs=1)
                nc.tensor.matmul(wt_ps[:], lhsT=warm_sb[:, 0:128], rhs=warm_sb[:],
                                 start=True, stop=True)

            # h = silu(x @ W_in.T + b_in)
            h_ps = psum_mm.tile([B, H], f32, tag="mma")
            nc.tensor.matmul(h_ps[:], lhsT=xT_sb[:], rhs=w_inT_sb[:], start=True, stop=True)
            h_pre = work_pool.tile([B, H], fp16, tag="hpre")
            nc.vector.tensor_add(h_pre[:], h_ps[:], b_in_sb[:])
            h_sb = const_pool.tile([B, H], f32)
            nc.scalar.activation(h_sb[:], h_pre[:], SILU, bias=0.0, scale=1.0)

            hT_sb = const_pool.tile([128, NSEG, B], fp16)
            for s in range(NSEG):
                transpose_to(psum_mm, h_sb[:, 128 * s:128 * (s + 1)], hT_sb[:, s, :])

            # K, Q, V (K first gates phase A; Q gates the q-broadcast)
            qkv_sb = [None, None, None]
            for m in (1, 0, 2):
                ps = psum_mm.tile([B, H], f32, tag="mma")
                for s in range(NSEG):
                    nc.tensor.matmul(ps[:], lhsT=hT_sb[:, s, :], rhs=aT_sb[m][:, s, :],
                                     start=(s == 0), stop=(s == NSEG - 1))
                pre = work_pool.tile([B, H], fp16, tag="qkvpre")
                nc.vector.tensor_add(pre[:], ps[:], bb_sb[m][:])
                t = const_pool.tile([B, H], f32, tag=f"qkv{m}")
                nc.scalar.activation(t[:], pre[:], SILU, bias=0.0, scale=1.0)
                qkv_sb[m] = t
                if m == 1:  # K -> ktil -> ktil8 roundtrip
                    kmax = work_pool.tile([B, 1], f32, tag="kmax")
                    nc.vector.tensor_reduce(kmax[:], t[:], axis=AX, op=MAX)
                    ktil = const_pool.tile([B, H], fp16)
                    nc.vector.tensor_scalar_sub(ktil[:], t[:], kmax[:])
                    nc.sync.dma_start(ktil_dram[:], ktil[:])
                    # stationary blocks for the S MMs: [(r,s), pair, j]
                    ktil8_sb = const_pool.tile([8, 32, 128], fp16)
                    ksrc = ktil_dram[:].rearrange(
                        "(pp r) (s j) -> (r s) pp j", r=2, s=NSEG)
                    nc.sync.dma_start(ktil8_sb[:], ksrc)
                elif m == 0:  # Q -> q16 -> broadcast
                    qfp = work_pool.tile([B, H], fp16, tag="qfp")
                    nc.vector.tensor_copy(qfp[:], t[:])
                    nc.sync.dma_start(q_dram[:], qfp[:])
                    qsrc = q_dram[:].rearrange("(gq x) i -> gq x i", x=4)
                    for quarter in range(4):
                        g0 = 4 * quarter
                        src = qsrc[g0:g0 + 4]
                        nc.sync.dma_start(
                            qb_all[0:NK, g0:g0 + 4, :, :],
                            src[:, 0:2, :].unsqueeze(0).broadcast_to((NK, 4, 2, H)))
                        nc.gpsimd.dma_start(
                            qb_all[BOT:NROW, g0:g0 + 4, :, :],
                            src[:, 2:4, :].unsqueeze(0).broadcast_to((NK, 4, 2, H)))
            q_sb, k_sb, v_sb = qkv_sb

            # vo_all[jj, g, s, (q,r,c)] = V[4g+2q+r, 128s+jj] (c=0) | 1 (c=1)
            vo_all = const_pool.tile([128, NG, NSEG, 8], fp16)
            nc.vector.memset(vo_all[:], 1.0)
            vT_ps = []
            for s in range(NSEG):
                pt = psum_mm.tile([128, B], f32, tag="tp")
                nc.tensor.transpose(pt[:], v_sb[:, 128 * s:128 * (s + 1)], eye_sb[:])
                vT_ps.append(pt)
                for qr in range(4):
                    nc.vector.tensor_copy(vo_all[:, :, s, 2 * qr], pt[:, qr::4])

        # =================== phase A ===================
        nden0 = const_pool.tile([32, 2, NP], f32, tag="nden0")
        nden1 = const_pool.tile([32, 2, NP], f32, tag="nden1")
        nden_h = [nden0, nden1]
        stage_nd = big_pool.tile([8, NG, 2, 2, NP], f32, tag="stagend")
        SFREE = 2 * NSEG * NP  # 384

        def drain_half(h):
            # nd garbage-cell filter: per (q, r) extract its diag col block
            for q in range(2):
                for r in range(2):
                    row = 4 * q + 2 * r
                    nc.sync.dma_start(
                        nden_h[h][2 * q + r:32:4, 0, :],
                        stage_nd[row:row + 1, 8 * h:8 * h + 8, q, r, :])
                    nc.sync.dma_start(
                        nden_h[h][2 * q + r:32:4, 1, :],
                        stage_nd[row + 1:row + 2, 8 * h:8 * h + 8, q, r, :])

        fn_half = [None, None]

        def node_math(h, psum_tp):
            dinv = work_pool.tile([32, NP], f32, tag="dinv")
            nc.vector.reciprocal(dinv[:], nden_h[h][:, 1, :])
            rn = work_pool.tile([32, NP], f32, tag="rn")
            nc.vector.tensor_mul(rn[:], nden_h[h][:, 0, :], dinv[:])
            fn = const_pool.tile([32, NP], f32, tag=f"fn{h}")
            nc.scalar.activation(fn[:], rn[:], SILU, bias=0.0, scale=1.0)
            fn_half[h] = fn
            slop = work_pool.tile([32, NK], f32, tag="slop")
            nc.vector.tensor_sub(slop[:], fn[:, 1:NP], fn[:, 0:NK])
            nc.vector.tensor_mul(slop[:], slop[:], invdt_sb[0:32, :])
            w_sb = work_pool.tile([32, NK], f32, tag="wsb")
            nc.vector.tensor_copy(w_sb[:, 0:1], slop[:, 0:1])
            nc.vector.tensor_sub(w_sb[:, 1:NK], slop[:, 1:NK], slop[:, 0:NK - 1])
            # transpose w once (PSUM base 0); scatter: sample b=32h+m has its
            # w col at flat 1024h+33m — top knot-rows if pair(b) even
            # (m%4<2, DVE copies) else bottom rows at partition 64 (DVE
            # can't shift partitions, so those go through tiny SBUF DMAs)
            pt_a = psum_tp.tile([128, B], f32, tag="tpwa")
            nc.tensor.transpose(pt_a[0:NK, 0:32], w_sb[:], eye_sb[0:32, 0:32])
            wTs = work_pool.tile([NK, 32], fp16, tag="wTs")
            nc.vector.tensor_copy(wTs[:], pt_a[0:NK, 0:32])
            for mm in range(4):
                c0 = 1024 * h + 33 * mm
                if mm < 2:
                    nc.vector.tensor_copy(
                        zbig[0:NK, c0:c0 + 33 * 28 + 1:33 * 4],
                        pt_a[0:NK, mm:32:4])
                else:
                    nc.gpsimd.dma_start(
                        zbig[BOT:NROW, c0:c0 + 33 * 28 + 1:33 * 4],
                        wTs[:, mm:32:4])

        with tc.tile_pool(name="pss", bufs=2, space="PSUM") as psum_s, \
             tc.tile_pool(name="psnd", bufs=2, space="PSUM") as psum_nd, \
             tc.tile_pool(name="pstp", bufs=1, space="PSUM") as psum_tp:
            for g in range(NG):
                s_ps = psum_s.tile([128, 2, 512], f32, tag="s")
                for q in range(2):
                    pp = 2 * g + q
                    nc.tensor.matmul(
                        s_ps[:, q, 0:SFREE],
                        lhsT=ktil8_sb[:, pp, :],
                        rhs=qhat2_sb[:],
                        start=True, stop=True)
                e_t = e_pool.tile([128, 2, 2, NSEG, NP], bf16, tag="e")
                nc.scalar.activation(
                    e_t[:].rearrange("j q r s n -> j q (r s n)"),
                    s_ps[:, :, 0:SFREE], EXP, bias=0.0, scale=1.0)
                nd_ps = psum_nd.tile([8, 2, 2, NP], f32, tag="nd")
                for s in range(NSEG):
                    nc.tensor.matmul(
                        nd_ps[:],
                        lhsT=vo_all[:, g, s, :],
                        rhs=e_t[:, :, :, s, :],
                        start=(s == 0), stop=(s == NSEG - 1))
                nc.vector.tensor_copy(stage_nd[:, g], nd_ps[:])
                if g == 7:
                    drain_half(0)
            drain_half(1)
            node_math(0, psum_tp)

        # =================== phase B ===================
        ctx_sb = const_pool.tile([B, H], f32, tag="ctx")
        with tc.tile_pool(name="pscx", bufs=2, space="PSUM") as psum_cx, \
             tc.tile_pool(name="pstpB", bufs=1, space="PSUM") as psum_tpB:
            ctx_ps = None
            for g in range(NG):
                if g == 2:
                    # second-half node math: needed from sample 32 (g=8);
                    # emitted here so its engine ops overlap early phase B
                    node_math(1, psum_tpB)
                bas = bas_pool.tile([NROW, 2, H], fp16, tag="bas")
                if g % 2 == 0:
                    nc.scalar.activation(bas[:], qb_all[:, g, :, :], RELU,
                                         bias=ntcol_sb[:], scale=1.0)
                else:
                    nc.vector.tensor_scalar(
                        bas[:], qb_all[:, g, :, :], tcol_sb[:], tcol_sb[:],
                        op0=MAX, op1=SUB)
                for bi in range(4):
                    b = 4 * g + bi
                    r = b % 2
                    if b % 32 == 0:
                        ctx_ps = psum_cx.tile([32, H], f32, tag="ctxps")
                    nc.tensor.matmul(
                        ctx_ps[:],
                        lhsT=zbig[:, 32 * b:32 * (b + 1)],
                        rhs=bas[:, r, :],
                        start=(b % 32 == 0), stop=(b % 32 == 31))
                    if b == 31:
                        nc.vector.tensor_scalar_add(
                            ctx_sb[0:32, :], ctx_ps[:], fn_half[0][:, 0:1])
                    elif b == 63:
                        cst = stage_pool.tile([32, H], f32, tag="cxst")
                        nc.vector.tensor_scalar_add(cst[:], ctx_ps[:], fn_half[1][:, 0:1])
                        nc.sync.dma_start(ctx_sb[32:64, :], cst[:])

            # ctx_T (F(t0) already added in the drains)
            ctxT_sb = work_pool.tile([128, NSEG, B], fp16, tag="ctxT")
            for s in range(NSEG):
                transpose_to(psum_tpB, ctx_sb[:, 128 * s:128 * (s + 1)], ctxT_sb[:, s, :])

            # y = silu(ctx @ W_out.T + b_out)
            y_ps = psum_tpB.tile([B, O], f32, tag="y")
            for s in range(NSEG):
                nc.tensor.matmul(y_ps[:], lhsT=ctxT_sb[:, s, :], rhs=w_outT_sb[:, s, :],
                                 start=(s == 0), stop=(s == NSEG - 1))
            y_pre = work_pool.tile([B, O], f32, tag="ypre")
            nc.vector.tensor_add(y_pre[:], y_ps[:], b_out_sb[:])
        y_sb = work_pool.tile([B, O], f32, tag="y")
        nc.scalar.activation(y_sb[:], y_pre[:], SILU, bias=0.0, scale=1.0)

        # ---- tail: block-diag quadratic form ----
        y2 = work_pool.tile([B, O], f32, tag="y2")
        nc.vector.tensor_mul(y2[:], y_sb[:], y_sb[:])
        m_sb = work_pool.tile([B, 5], f32, tag="m5")
        nc.vector.tensor_reduce(
            m_sb[:], y2[:].rearrange("p (a b) -> p a b", b=5), axis=AX,
            op=mybir.AluOpType.add)
        p2 = work_pool.tile([B, 2], f32, tag="p2")
        nc.vector.tensor_add(p2[:], y2[:, 0:3:2], y2[:, 1:4:2])
        c2 = work_pool.tile([B, 2], f32, tag="c2")
        nc.vector.tensor_mul(c2[:], y_sb[:, 0:2], y_sb[:, 2:4])
        cc = work_pool.tile([B, 1], f32, tag="cc")
        nc.vector.tensor_add(cc[:], c2[:, 0:1], c2[:, 1:2])
        mm12 = work_pool.tile([B, 1], f32, tag="mm12")
        nc.vector.tensor_add(mm12[:], m_sb[:, 1:2], m_sb[:, 2:3])
        mp = work_pool.tile([B, 2], f32, tag="mp")
        nc.vector.tensor_mul(mp[:], m_sb[:, 0:4:3], p2[:])
        acc = work_pool.tile([B, 1], f32, tag="acc")
        nc.vector.tensor_add(acc[:], mp[:, 0:1], mp[:, 1:2])
        acc2 = work_pool.tile([B, 1], f32, tag="acc2")
        nc.vector.tensor_mul(acc2[:], mm12[:], cc[:])
        acc3 = work_pool.tile([B, 1], f32, tag="acc3")
        nc.vector.tensor_add(acc3[:], acc[:], acc2[:])
        res = work_pool.tile([B, 1], f32, tag="res")
        nc.vector.tensor_add(res[:], acc3[:], m_sb[:, 4:5])
        nc.sync.dma_start(out_d[:], res[:])

    nc.finalize()
    return nc


def _host_inputs(x, W_in, b_in, Aq, Bq, Ak, Bk, Av, Bv, W_out, b_out):
    """Build the per-core input maps (shard x over batch; params replicated)."""
    f = lambda a: np.ascontiguousarray(a, dtype=np.float32)
    h16 = lambda a: np.ascontiguousarray(a, dtype=np.float16)
    t = _grid()  # already fp16-rounded values in f32
    # qhat2[(r,s), (r',s',p)] = t_p if r==r' and s==s' else 0
    qhat2 = np.zeros((8, 2 * NSEG * NP), np.float16)
    for r in range(2):
        for s in range(NSEG):
            col0 = (r * NSEG + s) * NP
            qhat2[r * NSEG + s, col0:col0 + NP] = t.astype(np.float16)
    tk = t[:NK]
    # rows 0-46: knots (top pair); 47-63: 1e4 filler (basis exactly 0);
    # 64-110: knots (bottom pair)
    tcol111 = np.full((BOT + NK, 1), 1e4, np.float32)
    tcol111[:NK, 0] = tk
    tcol111[BOT:, 0] = tk
    ntcol111 = -tcol111
    dt_ = t[1:] - t[:-1]
    common = {
        "w_inT": h16(W_in.T),
        "aqT": h16(Aq.T),
        "akT": h16(Ak.T),
        "avT": h16(Av.T),
        "b_in_bc": h16(np.broadcast_to(b_in, (B, H))),
        "bq_bc": h16(np.broadcast_to(Bq, (B, H))),
        "bk_bc": h16(np.broadcast_to(Bk, (B, H))),
        "bv_bc": h16(np.broadcast_to(Bv, (B, H))),
        "w_outT": h16(W_out.T),
        "b_out_bc": f(np.broadcast_to(b_out, (B, O))),
        "eye64": f(np.eye(B)),
        "qhat2": qhat2,
        "tcol111": tcol111,
        "ntcol111": ntcol111,
        "invdt_bc": f(np.broadcast_to(1.0 / dt_[:NK], (B, NK))),
    }
    xs = f(x).reshape(N_CORES, B, IN)
    return [dict(common, xT=h16(xs[i].T)) for i in range(N_CORES)]


def _get_nc():
    if "nc" not in _cache:
        _cache["nc"] = _build_nc()
    return _cache["nc"]


def run_spmd(in_maps, trace=False):
    from concourse.bass_utils import run_bass_kernel_spmd

    nc = _get_nc()
    res = run_bass_kernel_spmd(nc, in_maps, core_ids=list(range(N_CORES)), trace=trace)
    return res


def kernel(x, na=None, W_in=None, b_in=None, Aq=None, Bq=None, Ak=None, Bk=None,
           Av=None, Bv=None, W_out=None, b_out=None):
    in_maps = _host_inputs(x, W_in, b_in, Aq, Bq, Ak, Bk, Av, Bv, W_out, b_out)
    res = run_spmd(in_maps)
    out = np.concatenate([r["out"] for r in res.results], axis=0)
    return out.astype(np.float32)


# revision 24
# speedup vs baseline: 1.2924x; 1.2924x over previous
"""Trainium2 Bass kernel for nn_Att_H (rank-1 attention MLP), 8-core data parallel.

Math (per sample b):
  h = silu(x @ W_in.T + b_in); Q,K,V = silu(h @ A*.T + B*)
  scores[i,j] = Q[i]*K[j]; attn = softmax_j; ctx = silu(attn @ V)
  y = silu(ctx @ W_out.T + b_out); out = quad-form tail on y.

Rank-1 scores => ctx_i = F(Q_i) where F(q) = silu(num(q)/den(q)),
num(q) = sum_j V_j e^{q ktil_j}, den(q) = sum_j e^{q ktil_j}
(ktil = K - Kmax <= 0). F is evaluated exactly at NP=48 grid nodes and
interpolated piecewise-linearly at the 512 Q_i via a relu-basis matmul:
  F(q) = F(t_0) + sum_p w_p relu(q - t_p),  w = 2nd differences of slopes.

Layout tricks vs the v0 kernel (137us):
- NP=96 -> 48 with a retuned two-piece grid (offline sim err 6.3e-3 vs
  gate 2e-2).
- Phase A pair-packed: one S matmul per sample pair (stationary
  ktil8 [8,128] = nearly free LDWEIGHTS), one Exp per 2 pairs, and
  nd matmuls seg-packed over 2 pairs ([128,8] stationary, garbage
  cross-cells discarded at drain).
- 2-pair knot packing: both pairs of a 4-sample group share one
  [94, 2, 512] basis tile (47 knots each half), halving elementwise
  basis work; built alternately on ACT (relu bias trick) and DVE
  (max-sub), sourced from 8 bulk q-broadcast DMAs (3MB total).
- nd drains: DVE-staged, then 16 strided DMAs into [64, 2, 48].
- Node math split in halves so phase B can start before phase A ends.
- Big fp16 warm-up matmuls + early param DMAs attack the PE p-state.
"""

import sys
import numpy as np

for _p in ("/opt/trn_rl_repo", "/opt/trn_rl_repo/concourse"):
    if _p not in sys.path:
        sys.path.append(_p)

B_GLOBAL = 512
N_CORES = 8
B = B_GLOBAL // N_CORES  # 64 samples per core
IN = 128
H = 512
O = 25
NSEG = H // 128  # 4

# PWL grid: 48 nodes, two-piece linear (dense low where Q clusters)
GRID_LO, GRID_MID, GRID_HI = -0.36, 6.0, 44.0
N_LO = 36
NP = 48
NK = NP - 1   # 47 knots
NG = 16       # groups of 4 samples (2 pairs)
BOT = 64      # base partition of the second pair's knot block (HW requires
              # matmul/transpose base partitions in {0, 32, 64})
NROW = BOT + NK  # 111


def _grid():
    t = np.concatenate([
        np.linspace(GRID_LO, GRID_MID, N_LO, endpoint=False),
        np.linspace(GRID_MID, GRID_HI, NP - N_LO),
    ]).astype(np.float16).astype(np.float32)
    return t


_cache = {}


def _build_nc():
    from contextlib import ExitStack

    import concourse.bass as bass
    import concourse.tile as tile
    from concourse import bacc, mybir

    f32 = mybir.dt.float32
    bf16 = mybir.dt.bfloat16
    fp16 = mybir.dt.float16
    EXP = mybir.ActivationFunctionType.Exp
    SILU = mybir.ActivationFunctionType.Silu
    RELU = mybir.ActivationFunctionType.Relu
    AX = mybir.AxisListType.X
    MAX = mybir.AluOpType.max
    SUB = mybir.AluOpType.subtract

    nc = bacc.Bacc()
    x_d = nc.declare_dram_parameter("xT", [IN, B], fp16, False)
    w_inT_d = nc.declare_dram_parameter("w_inT", [IN, H], fp16, False)
    aT_d = [nc.declare_dram_parameter(f"a{m}T", [H, H], fp16, False) for m in "qkv"]
    b_in_d = nc.declare_dram_parameter("b_in_bc", [B, H], fp16, False)
    bb_d = [nc.declare_dram_parameter(f"b{m}_bc", [B, H], fp16, False) for m in "qkv"]
    w_outT_d = nc.declare_dram_parameter("w_outT", [H, O], fp16, False)
    b_out_d = nc.declare_dram_parameter("b_out_bc", [B, O], f32, False)
    eye_d = nc.declare_dram_parameter("eye64", [B, B], f32, False)
    qhat2_d = nc.declare_dram_parameter("qhat2", [8, 2 * NSEG * NP], fp16, False)
    tcol_d = nc.declare_dram_parameter("tcol111", [NROW, 1], f32, False)
    ntcol_d = nc.declare_dram_parameter("ntcol111", [NROW, 1], f32, False)
    invdt_d = nc.declare_dram_parameter("invdt_bc", [B, NK], f32, False)
    out_d = nc.declare_dram_parameter("out", [B, 1], f32, True)
    q_dram = nc.dram_tensor("q_scratch", [B, H], fp16)
    ktil_dram = nc.dram_tensor("ktil_scratch", [B, H], fp16)

    with tile.TileContext(nc) as tc, ExitStack() as ctx:
        const_pool = ctx.enter_context(tc.tile_pool(name="const", bufs=1))
        big_pool = ctx.enter_context(tc.tile_pool(name="big", bufs=1))
        work_pool = ctx.enter_context(tc.tile_pool(name="work", bufs=2))
        stage_pool = ctx.enter_context(tc.tile_pool(name="stg", bufs=2))
        e_pool = ctx.enter_context(tc.tile_pool(name="et", bufs=3))
        bas_pool = ctx.enter_context(tc.tile_pool(name="bas", bufs=3))

        # ---- param loads: early-needed first on sync, rest on gpsimd ----
        xT_sb = const_pool.tile([IN, B], fp16)
        nc.sync.dma_start(xT_sb[:], x_d[:])
        w_inT_sb = const_pool.tile([IN, H], fp16)
        nc.sync.dma_start(w_inT_sb[:], w_inT_d[:])
        b_in_sb = const_pool.tile([B, H], fp16)
        nc.sync.dma_start(b_in_sb[:], b_in_d[:])
        eye_sb = const_pool.tile([B, B], f32)
        nc.sync.dma_start(eye_sb[:], eye_d[:])
        aT_sb = [None, None, None]
        for mi in (0, 1, 2):  # Q first: it gates the q-broadcast
            t = big_pool.tile([128, NSEG, H], fp16, tag=f"aT{mi}")
            src_r = aT_d[mi][:].rearrange("(s p) i -> p s i", p=128)
            nc.sync.dma_start(t[:, 0:2, :], src_r[:, 0:2, :])
            nc.sync.dma_start(t[:, 2:4, :], src_r[:, 2:4, :])
            aT_sb[mi] = t
        bb_sb = []
        for mi, d in enumerate(bb_d):
            t = const_pool.tile([B, H], fp16, tag=f"bb{mi}")
            nc.sync.dma_start(t[:], d[:])
            bb_sb.append(t)
        qhat2_sb = const_pool.tile([8, 2 * NSEG * NP], fp16)
        nc.sync.dma_start(qhat2_sb[:], qhat2_d[:])
        tcol_sb = const_pool.tile([NROW, 1], f32)
        nc.sync.dma_start(tcol_sb[:], tcol_d[:])
        ntcol_sb = const_pool.tile([NROW, 1], f32)
        nc.sync.dma_start(ntcol_sb[:], ntcol_d[:])
        invdt_sb = const_pool.tile([B, NK], f32)
        nc.sync.dma_start(invdt_sb[:], invdt_d[:])
        w_outT_sb = const_pool.tile([128, NSEG, O], fp16)
        nc.sync.dma_start(w_outT_sb[:], w_outT_d[:].rearrange("(s p) o -> p s o", p=128))
        b_out_sb = const_pool.tile([B, O], f32)
        nc.sync.dma_start(b_out_sb[:], b_out_d[:])

        # big SBUF tensors (rows 47-63 of the knot blocks are dead filler:
        # tcol there is 1e4 so the basis is exactly 0, zbig rows are 0; the
        # bottom q-broadcast covers those rows so no memset is needed)
        qb_all = big_pool.tile([NROW, NG, 2, H], fp16, tag="qball")
        zbig = const_pool.tile([NROW, B * 32 + 64], fp16, tag="zbig")
        nc.vector.memset(zbig[:], 0.0)
        warm_sb = const_pool.tile([128, H], fp16, tag="warm")
        nc.vector.memset(warm_sb[:], 0.0)

        def transpose_to(pool, src_ap, dst_ap):
            """[p0<=64, f<=128] SBUF -> [f, p0] SBUF via PE transpose."""
            p0 = src_ap.shape[0]
            f = src_ap.shape[-1]
            pt = pool.tile([128, B], f32, tag="tp")
            nc.tensor.transpose(pt[0:f, 0:p0], src_ap, eye_sb[0:p0, 0:p0])
            nc.vector.tensor_copy(dst_ap, pt[0:f, 0:p0])

        # =================== phase 0 ===================
        with tc.tile_pool(name="ps0", bufs=2, space="PSUM") as psum_mm:
            # PE warm-up: fat fp16 MMs to push the p-state up while params load
            for wi in range(14):
                wt_ps = psum_mm.tile([128, H], f32, tag="warm", bufs=1)
                nc.tensor.matmul(wt_ps[:], lhsT=warm_sb[:, 0:128], rhs=warm_sb[:],
                                 start=True, stop=True)

            # h = silu(x @ W_in.T + b_in)
            h_ps = psum_mm.tile([B, H], f32, tag="mma")
            nc.tensor.matmul(h_ps[:], lhsT=xT_sb[:], rhs=w_inT_sb[:], start=True, stop=True)
            h_pre = work_pool.tile([B, H], fp16, tag="hpre")
            nc.vector.tensor_add(h_pre[:], h_ps[:], b_in_sb[:])
            h_sb = const_pool.tile([B, H], f32)
            nc.scalar.activation(h_sb[:], h_pre[:], SILU, bias=0.0, scale=1.0)

            hT_sb = const_pool.tile([128, NSEG, B], fp16)
            for s in range(NSEG):
                transpose_to(psum_mm, h_sb[:, 128 * s:128 * (s + 1)], hT_sb[:, s, :])

            # Q, K, V (Q first gates the q-broadcast; K gates phase A)
            qkv_sb = [None, None, None]
            for m in (0, 1, 2):
                ps = psum_mm.tile([B, H], f32, tag="mma")
                for s in range(NSEG):
                    nc.tensor.matmul(ps[:], lhsT=hT_sb[:, s, :], rhs=aT_sb[m][:, s, :],
                                     start=(s == 0), stop=(s == NSEG - 1))
                pre = work_pool.tile([B, H], fp16, tag="qkvpre")
                nc.vector.tensor_add(pre[:], ps[:], bb_sb[m][:])
                t = const_pool.tile([B, H], f32, tag=f"qkv{m}")
                nc.scalar.activation(t[:], pre[:], SILU, bias=0.0, scale=1.0)
                qkv_sb[m] = t
                if m == 1:  # K -> ktil -> ktil8 roundtrip
                    kmax = work_pool.tile([B, 1], f32, tag="kmax")
                    nc.vector.tensor_reduce(kmax[:], t[:], axis=AX, op=MAX)
                    ktil = const_pool.tile([B, H], fp16)
                    nc.vector.tensor_scalar_sub(ktil[:], t[:], kmax[:])
                    nc.sync.dma_start(ktil_dram[:], ktil[:])
                    # stationary blocks for the S MMs: [(r,s), pair, j]
                    ktil8_sb = const_pool.tile([8, 32, 128], fp16)
                    ksrc = ktil_dram[:].rearrange(
                        "(pp r) (s j) -> (r s) pp j", r=2, s=NSEG)
                    nc.sync.dma_start(ktil8_sb[:], ksrc)
                elif m == 0:  # Q -> q16 -> broadcast
                    qfp = work_pool.tile([B, H], fp16, tag="qfp")
                    nc.vector.tensor_copy(qfp[:], t[:])
                    nc.sync.dma_start(q_dram[:], qfp[:])
                    # q broadcasts: gpsimd SWDGE only (sync HWDGE takes
                    # 9-27us to generate a broadcast pattern and serializes
                    # the queue); bottom DMA also fills dead rows 47-63
                    qsrc = q_dram[:].rearrange("(gq x) i -> gq x i", x=4)
                    for quarter in range(4):
                        g0 = 4 * quarter
                        src = qsrc[g0:g0 + 4]
                        nc.gpsimd.dma_start(
                            qb_all[0:NK, g0:g0 + 4, :, :],
                            src[:, 0:2, :].unsqueeze(0).broadcast_to((NK, 4, 2, H)))
                        nc.gpsimd.dma_start(
                            qb_all[NK:NROW, g0:g0 + 4, :, :],
                            src[:, 2:4, :].unsqueeze(0).broadcast_to((NROW - NK, 4, 2, H)))
            q_sb, k_sb, v_sb = qkv_sb

            # vo_all[jj, g, s, (q,r,c)] = V[4g+2q+r, 128s+jj] (c=0) | 1 (c=1)
            vo_all = const_pool.tile([128, NG, NSEG, 8], fp16)
            nc.vector.memset(vo_all[:], 1.0)
            vT_ps = []
            for s in range(NSEG):
                pt = psum_mm.tile([128, B], f32, tag="tp")
                nc.tensor.transpose(pt[:], v_sb[:, 128 * s:128 * (s + 1)], eye_sb[:])
                vT_ps.append(pt)
                for qr in range(4):
                    nc.vector.tensor_copy(vo_all[:, :, s, 2 * qr], pt[:, qr::4])

        # =================== phase A ===================
        nden0 = const_pool.tile([32, 2, NP], f32, tag="nden0")
        nden1 = const_pool.tile([32, 2, NP], f32, tag="nden1")
        nden_h = [nden0, nden1]
        stage_nd = big_pool.tile([8, NG, 2, 2, NP], f32, tag="stagend")
        SFREE = 2 * NSEG * NP  # 384

        def drain_half(h):
            # nd garbage-cell filter: per (q, r) extract its diag col block
            for q in range(2):
                for r in range(2):
                    row = 4 * q + 2 * r
                    nc.sync.dma_start(
                        nden_h[h][2 * q + r:32:4, 0, :],
                        stage_nd[row:row + 1, 8 * h:8 * h + 8, q, r, :])
                    nc.sync.dma_start(
                        nden_h[h][2 * q + r:32:4, 1, :],
                        stage_nd[row + 1:row + 2, 8 * h:8 * h + 8, q, r, :])

        fn_half = [None, None]

        def node_math(h, psum_tp):
            dinv = work_pool.tile([32, NP], f32, tag="dinv")
            nc.vector.reciprocal(dinv[:], nden_h[h][:, 1, :])
            rn = work_pool.tile([32, NP], f32, tag="rn")
            nc.vector.tensor_mul(rn[:], nden_h[h][:, 0, :], dinv[:])
            fn = const_pool.tile([32, NP], f32, tag=f"fn{h}")
            nc.scalar.activation(fn[:], rn[:], SILU, bias=0.0, scale=1.0)
            fn_half[h] = fn
            slop = work_pool.tile([32, NK], f32, tag="slop")
            nc.vector.tensor_sub(slop[:], fn[:, 1:NP], fn[:, 0:NK])
            nc.vector.tensor_mul(slop[:], slop[:], invdt_sb[0:32, :])
            w_sb = work_pool.tile([32, NK], f32, tag="wsb")
            nc.vector.tensor_copy(w_sb[:, 0:1], slop[:, 0:1])
            nc.vector.tensor_sub(w_sb[:, 1:NK], slop[:, 1:NK], slop[:, 0:NK - 1])
            # transpose w once (PSUM base 0); scatter: sample b=32h+m has its
            # w col at flat 1024h+33m — top knot-rows if pair(b) even
            # (m%4<2, DVE copies) else bottom rows at partition 64 (DVE
            # can't shift partitions, so those go through tiny SBUF DMAs)
            pt_a = psum_tp.tile([128, B], f32, tag="tpwa")
            nc.tensor.transpose(pt_a[0:NK, 0:32], w_sb[:], eye_sb[0:32, 0:32])
            wTs = work_pool.tile([NK, 32], fp16, tag="wTs")
            nc.vector.tensor_copy(wTs[:], pt_a[0:NK, 0:32])
            for mm in range(4):
                c0 = 1024 * h + 33 * mm
                if mm < 2:
                    nc.vector.tensor_copy(
                        zbig[0:NK, c0:c0 + 33 * 28 + 1:33 * 4],
                        pt_a[0:NK, mm:32:4])
                else:
                    nc.gpsimd.dma_start(
                        zbig[BOT:NROW, c0:c0 + 33 * 28 + 1:33 * 4],
                        wTs[:, mm:32:4])

        with tc.tile_pool(name="pss", bufs=2, space="PSUM") as psum_s, \
             tc.tile_pool(name="psnd", bufs=2, space="PSUM") as psum_nd, \
             tc.tile_pool(name="pstp", bufs=1, space="PSUM") as psum_tp:
            for g in range(NG):
                s_ps = psum_s.tile([128, 2, 512], f32, tag="s")
                for q in range(2):
                    pp = 2 * g + q
                    nc.tensor.matmul(
                        s_ps[:, q, 0:SFREE],
                        lhsT=ktil8_sb[:, pp, :],
                        rhs=qhat2_sb[:],
                        start=True, stop=True)
                e_t = e_pool.tile([128, 2, 2, NSEG, NP], bf16, tag="e")
                nc.scalar.activation(
                    e_t[:].rearrange("j q r s n -> j q (r s n)"),
                    s_ps[:, :, 0:SFREE], EXP, bias=0.0, scale=1.0)
                nd_ps = psum_nd.tile([8, 2, 2, NP], f32, tag="nd")
                for s in range(NSEG):
                    nc.tensor.matmul(
                        nd_ps[:],
                        lhsT=vo_all[:, g, s, :],
                        rhs=e_t[:, :, :, s, :],
                        start=(s == 0), stop=(s == NSEG - 1))
                nc.vector.tensor_copy(stage_nd[:, g], nd_ps[:])
                if g == 7:
                    drain_half(0)
            drain_half(1)
            node_math(0, psum_tp)

        # =================== phase B ===================
        ctx_sb = const_pool.tile([B, H], f32, tag="ctx")
        with tc.tile_pool(name="pscx", bufs=2, space="PSUM") as psum_cx, \
             tc.tile_pool(name="pstpB", bufs=1, space="PSUM") as psum_tpB:
            ctx_ps = None
            for g in range(NG):
                if g == 2:
                    # second-half node math: needed from sample 32 (g=8);
                    # emitted here so its engine ops overlap early phase B
                    node_math(1, psum_tpB)
                bas = bas_pool.tile([NROW, 2, H], fp16, tag="bas")
                if g % 3 == 2:
                    nc.scalar.activation(bas[:], qb_all[:, g, :, :], RELU,
                                         bias=ntcol_sb[:], scale=1.0)
                else:
                    nc.vector.tensor_scalar(
                        bas[:], qb_all[:, g, :, :], tcol_sb[:], tcol_sb[:],
                        op0=MAX, op1=SUB)
                for bi in range(4):
                    b = 4 * g + bi
                    r = b % 2
                    if b % 32 == 0:
                        ctx_ps = psum_cx.tile([32, H], f32, tag="ctxps")
                    nc.tensor.matmul(
                        ctx_ps[:],
                        lhsT=zbig[:, 32 * b:32 * (b + 1)],
                        rhs=bas[:, r, :],
                        start=(b % 32 == 0), stop=(b % 32 == 31))
                    if b == 31:
                        nc.vector.tensor_scalar_add(
                            ctx_sb[0:32, :], ctx_ps[:], fn_half[0][:, 0:1])
                    elif b == 63:
                        cst = stage_pool.tile([32, H], f32, tag="cxst")
                        nc.vector.tensor_scalar_add(cst[:], ctx_ps[:], fn_half[1][:, 0:1])
                        nc.sync.dma_start(ctx_sb[32:64, :], cst[:])

            # ctx_T (F(t0) already added in the drains)
            ctxT_sb = work_pool.tile([128, NSEG, B], fp16, tag="ctxT")
            for s in range(NSEG):
                transpose_to(psum_tpB, ctx_sb[:, 128 * s:128 * (s + 1)], ctxT_sb[:, s, :])

            # y = silu(ctx @ W_out.T + b_out)
            y_ps = psum_tpB.tile([B, O], f32, tag="y")
            for s in range(NSEG):
                nc.tensor.matmul(y_ps[:], lhsT=ctxT_sb[:, s, :], rhs=w_outT_sb[:, s, :],
                                 start=(s == 0), stop=(s == NSEG - 1))
            y_pre = work_pool.tile([B, O], f32, tag="ypre")
            nc.vector.tensor_add(y_pre[:], y_ps[:], b_out_sb[:])
        y_sb = work_pool.tile([B, O], f32, tag="y")
        nc.scalar.activation(y_sb[:], y_pre[:], SILU, bias=0.0, scale=1.0)

        # ---- tail: block-diag quadratic form ----
        y2 = work_pool.tile([B, O], f32, tag="y2")
        nc.vector.tensor_mul(y2[:], y_sb[:], y_sb[:])
        m_sb = work_pool.tile([B, 5], f32, tag="m5")
        nc.vector.tensor_reduce(
            m_sb[:], y2[:].rearrange("p (a b) -> p a b", b=5), axis=AX,
            op=mybir.AluOpType.add)
        p2 = work_pool.tile([B, 2], f32, tag="p2")
        nc.vector.tensor_add(p2[:], y2[:, 0:3:2], y2[:, 1:4:2])
        c2 = work_pool.tile([B, 2], f32, tag="c2")
        nc.vector.tensor_mul(c2[:], y_sb[:, 0:2], y_sb[:, 2:4])
        cc = work_pool.tile([B, 1], f32, tag="cc")
        nc.vector.tensor_add(cc[:], c2[:, 0:1], c2[:, 1:2])
        mm12 = work_pool.tile([B, 1], f32, tag="mm12")
        nc.vector.tensor_add(mm12[:], m_sb[:, 1:2], m_sb[:, 2:3])
        mp = work_pool.tile([B, 2], f32, tag="mp")
        nc.vector.tensor_mul(mp[:], m_sb[:, 0:4:3], p2[:])
        acc = work_pool.tile([B, 1], f32, tag="acc")
        nc.vector.tensor_add(acc[:], mp[:, 0:1], mp[:, 1:2])
        acc2 = work_pool.tile([B, 1], f32, tag="acc2")
        nc.vector.tensor_mul(acc2[:], mm12[:], cc[:])
        acc3 = work_pool.tile([B, 1], f32, tag="acc3")
        nc.vector.tensor_add(acc3[:], acc[:], acc2[:])
        res = work_pool.tile([B, 1], f32, tag="res")
        nc.vector.tensor_add(res[:], acc3[:], m_sb[:, 4:5])
        nc.sync.dma_start(out_d[:], res[:])

    nc.finalize()
    return nc


def _host_inputs(x, W_in, b_in, Aq, Bq, Ak, Bk, Av, Bv, W_out, b_out):
    """Build the per-core input maps (shard x over batch; params replicated)."""
    f = lambda a: np.ascontiguousarray(a, dtype=np.float32)
    h16 = lambda a: np.ascontiguousarray(a, dtype=np.float16)
    t = _grid()  # already fp16-rounded values in f32
    # qhat2[(r,s), (r',s',p)] = t_p if r==r' and s==s' else 0
    qhat2 = np.zeros((8, 2 * NSEG * NP), np.float16)
    for r in range(2):
        for s in range(NSEG):
            col0 = (r * NSEG + s) * NP
            qhat2[r * NSEG + s, col0:col0 + NP] = t.astype(np.float16)
    tk = t[:NK]
    # rows 0-46: knots (top pair); 47-63: 1e4 filler (basis exactly 0);
    # 64-110: knots (bottom pair)
    tcol111 = np.full((BOT + NK, 1), 1e4, np.float32)
    tcol111[:NK, 0] = tk
    tcol111[BOT:, 0] = tk
    ntcol111 = -tcol111
    dt_ = t[1:] - t[:-1]
    common = {
        "w_inT": h16(W_in.T),
        "aqT": h16(Aq.T),
        "akT": h16(Ak.T),
        "avT": h16(Av.T),
        "b_in_bc": h16(np.broadcast_to(b_in, (B, H))),
        "bq_bc": h16(np.broadcast_to(Bq, (B, H))),
        "bk_bc": h16(np.broadcast_to(Bk, (B, H))),
        "bv_bc": h16(np.broadcast_to(Bv, (B, H))),
        "w_outT": h16(W_out.T),
        "b_out_bc": f(np.broadcast_to(b_out, (B, O))),
        "eye64": f(np.eye(B)),
        "qhat2": qhat2,
        "tcol111": tcol111,
        "ntcol111": ntcol111,
        "invdt_bc": f(np.broadcast_to(1.0 / dt_[:NK], (B, NK))),
    }
    xs = f(x).reshape(N_CORES, B, IN)
    return [dict(common, xT=h16(xs[i].T)) for i in range(N_CORES)]


def _get_nc():
    if "nc" not in _cache:
        _cache["nc"] = _build_nc()
    return _cache["nc"]


def run_spmd(in_maps, trace=False):
    from concourse.bass_utils import run_bass_kernel_spmd

    nc = _get_nc()
    res = run_bass_kernel_spmd(nc, in_maps, core_ids=list(range(N_CORES)), trace=trace)
    return res


def kernel(x, na=None, W_in=None, b_in=None, Aq=None, Bq=None, Ak=None, Bk=None,
           Av=None, Bv=None, W_out=None, b_out=None):
    in_maps = _host_inputs(x, W_in, b_in, Aq, Bq, Ak, Bk, Av, Bv, W_out, b_out)
    res = run_spmd(in_maps)
    out = np.concatenate([r["out"] for r in res.results], axis=0)
    return out.astype(np.float32)


# revision 28
# speedup vs baseline: 1.5078x; 1.1666x over previous
"""Trainium2 Bass kernel for nn_Att_H (rank-1 attention MLP), 8-core data parallel.

Math (per sample b):
  h = silu(x @ W_in.T + b_in); Q,K,V = silu(h @ A*.T + B*)
  scores[i,j] = Q[i]*K[j]; attn = softmax_j; ctx = silu(attn @ V)
  y = silu(ctx @ W_out.T + b_out); out = quad-form tail on y.

Rank-1 scores => ctx_i = F(Q_i) where F(q) = silu(num(q)/den(q)),
num(q) = sum_j V_j e^{q ktil_j}, den(q) = sum_j e^{q ktil_j}
(ktil = K - Kmax <= 0). F is evaluated exactly at NP=48 grid nodes and
interpolated piecewise-linearly at the 512 Q_i via a relu-basis matmul:
  F(q) = F(t_0) + sum_p w_p relu(q - t_p),  w = 2nd differences of slopes.

Layout tricks vs the v0 kernel (137us):
- NP=96 -> 48 with a retuned two-piece grid (offline sim err 6.3e-3 vs
  gate 2e-2).
- Phase A pair-packed: one S matmul per sample pair (stationary
  ktil8 [8,128] = nearly free LDWEIGHTS), one Exp per 2 pairs, and
  nd matmuls seg-packed over 2 pairs ([128,8] stationary, garbage
  cross-cells discarded at drain).
- 2-pair knot packing: both pairs of a 4-sample group share one
  [94, 2, 512] basis tile (47 knots each half), halving elementwise
  basis work; built alternately on ACT (relu bias trick) and DVE
  (max-sub), sourced from 8 bulk q-broadcast DMAs (3MB total).
- nd drains: DVE-staged, then 16 strided DMAs into [64, 2, 48].
- Node math split in halves so phase B can start before phase A ends.
- Big fp16 warm-up matmuls + early param DMAs attack the PE p-state.
"""

import sys
import numpy as np

for _p in ("/opt/trn_rl_repo", "/opt/trn_rl_repo/concourse"):
    if _p not in sys.path:
        sys.path.append(_p)

B_GLOBAL = 512
N_CORES = 8
B = B_GLOBAL // N_CORES  # 64 samples per core
IN = 128
H = 512
O = 25
NSEG = H // 128  # 4

# PWL grid: 48 nodes, two-piece linear (dense low where Q clusters)
GRID_LO, GRID_MID, GRID_HI = -0.36, 6.0, 44.0
N_LO = 36
NP = 48
NK = NP - 1   # 47 knots
NG = 16       # groups of 4 samples (2 pairs)
BOT = 64      # base partition of the second pair's knot block (HW requires
              # matmul/transpose base partitions in {0, 32, 64})
NROW = BOT + NK  # 111


def _grid():
    t = np.concatenate([
        np.linspace(GRID_LO, GRID_MID, N_LO, endpoint=False),
        np.linspace(GRID_MID, GRID_HI, NP - N_LO),
    ]).astype(np.float16).astype(np.float32)
    return t


_cache = {}


def _build_nc():
    from contextlib import ExitStack

    import concourse.bass as bass
    import concourse.tile as tile
    from concourse import bacc, mybir

    f32 = mybir.dt.float32
    bf16 = mybir.dt.bfloat16
    fp16 = mybir.dt.float16
    EXP = mybir.ActivationFunctionType.Exp
    SILU = mybir.ActivationFunctionType.Silu
    RELU = mybir.ActivationFunctionType.Relu
    AX = mybir.AxisListType.X
    MAX = mybir.AluOpType.max
    SUB = mybir.AluOpType.subtract

    nc = bacc.Bacc()
    x_d = nc.declare_dram_parameter("xT", [IN, B], fp16, False)
    w_inT_d = nc.declare_dram_parameter("w_inT", [IN, H], fp16, False)
    aT_d = [nc.declare_dram_parameter(f"a{m}T", [H, H], fp16, False) for m in "qkv"]
    b_in_d = nc.declare_dram_parameter("b_in_bc", [B, H], fp16, False)
    bb_d = [nc.declare_dram_parameter(f"b{m}_bc", [B, H], fp16, False) for m in "qkv"]
    w_outT_d = nc.declare_dram_parameter("w_outT", [H, O], fp16, False)
    b_out_d = nc.declare_dram_parameter("b_out_bc", [B, O], f32, False)
    eye_d = nc.declare_dram_parameter("eye64", [B, B], f32, False)
    qhat2_d = nc.declare_dram_parameter("qhat2", [8, 2 * NSEG * NP], fp16, False)
    tcol_d = nc.declare_dram_parameter("tcol111", [NROW, 1], f32, False)
    ntcol_d = nc.declare_dram_parameter("ntcol111", [NROW, 1], f32, False)
    invdt_d = nc.declare_dram_parameter("invdt_bc", [B, NK], f32, False)
    out_d = nc.declare_dram_parameter("out", [B, 1], f32, True)
    q_dram = nc.dram_tensor("q_scratch", [B, H], fp16)
    ktil_dram = nc.dram_tensor("ktil_scratch", [B, H], fp16)

    with tile.TileContext(nc) as tc, ExitStack() as ctx:
        const_pool = ctx.enter_context(tc.tile_pool(name="const", bufs=1))
        big_pool = ctx.enter_context(tc.tile_pool(name="big", bufs=1))
        work_pool = ctx.enter_context(tc.tile_pool(name="work", bufs=2))
        stage_pool = ctx.enter_context(tc.tile_pool(name="stg", bufs=2))
        e_pool = ctx.enter_context(tc.tile_pool(name="et", bufs=3))
        bas_pool = ctx.enter_context(tc.tile_pool(name="bas", bufs=3))

        # ---- param loads: early-needed first on sync, rest on gpsimd ----
        xT_sb = const_pool.tile([IN, B], fp16)
        nc.sync.dma_start(xT_sb[:], x_d[:])
        w_inT_sb = const_pool.tile([IN, H], fp16)
        nc.sync.dma_start(w_inT_sb[:], w_inT_d[:])
        b_in_sb = const_pool.tile([B, H], fp16)
        nc.sync.dma_start(b_in_sb[:], b_in_d[:])
        eye_sb = const_pool.tile([B, B], f32)
        nc.sync.dma_start(eye_sb[:], eye_d[:])
        aT_sb = [None, None, None]
        for mi in (0, 1, 2):  # Q first: it gates the q-broadcast
            t = big_pool.tile([128, NSEG, H], fp16, tag=f"aT{mi}")
            src_r = aT_d[mi][:].rearrange("(s p) i -> p s i", p=128)
            nc.sync.dma_start(t[:, 0:2, :], src_r[:, 0:2, :])
            nc.sync.dma_start(t[:, 2:4, :], src_r[:, 2:4, :])
            aT_sb[mi] = t
        bb_sb = []
        for mi, d in enumerate(bb_d):
            t = const_pool.tile([B, H], fp16, tag=f"bb{mi}")
            nc.sync.dma_start(t[:], d[:])
            bb_sb.append(t)
        qhat2_sb = const_pool.tile([8, 2 * NSEG * NP], fp16)
        nc.sync.dma_start(qhat2_sb[:], qhat2_d[:])

        # big SBUF tensors (rows 47-63 of the knot blocks are dead filler:
        # tcol there is 1e4 so the basis is exactly 0, zbig rows are 0; the
        # bottom q-broadcast covers those rows so no memset is needed)
        qb_all = big_pool.tile([NROW, NG, 2, H], fp16, tag="qball")
        zbig = const_pool.tile([NROW, B * 32 + 64], fp16, tag="zbig")
        nc.vector.memset(zbig[:], 0.0)
        warm_sb = const_pool.tile([128, H], fp16, tag="warm")
        nc.vector.memset(warm_sb[:], 0.0)

        def transpose_to(pool, src_ap, dst_ap):
            """[p0<=64, f<=128] SBUF -> [f, p0] SBUF via PE transpose."""
            p0 = src_ap.shape[0]
            f = src_ap.shape[-1]
            pt = pool.tile([128, B], f32, tag="tp")
            nc.tensor.transpose(pt[0:f, 0:p0], src_ap, eye_sb[0:p0, 0:p0])
            nc.vector.tensor_copy(dst_ap, pt[0:f, 0:p0])

        # =================== phase 0 ===================
        with tc.tile_pool(name="ps0", bufs=2, space="PSUM") as psum_mm:
            # PE warm-up: fat fp16 MMs to push the p-state up while params load
            for wi in range(14):
                wt_ps = psum_mm.tile([128, H], f32, tag="warm", bufs=1)
                nc.tensor.matmul(wt_ps[:], lhsT=warm_sb[:, 0:128], rhs=warm_sb[:],
                                 start=True, stop=True)

            # h = silu(x @ W_in.T + b_in)
            h_ps = psum_mm.tile([B, H], f32, tag="mma")
            nc.tensor.matmul(h_ps[:], lhsT=xT_sb[:], rhs=w_inT_sb[:], start=True, stop=True)
            h_pre = work_pool.tile([B, H], fp16, tag="hpre")
            nc.vector.tensor_add(h_pre[:], h_ps[:], b_in_sb[:])
            h_sb = const_pool.tile([B, H], f32)
            nc.scalar.activation(h_sb[:], h_pre[:], SILU, bias=0.0, scale=1.0)

            hT_sb = const_pool.tile([128, NSEG, B], fp16)
            for s in range(NSEG):
                transpose_to(psum_mm, h_sb[:, 128 * s:128 * (s + 1)], hT_sb[:, s, :])

            # Q, K, V (Q first gates the q-broadcast; K gates phase A)
            qkv_sb = [None, None, None]
            for m in (0, 1, 2):
                ps = psum_mm.tile([B, H], f32, tag="mma")
                for s in range(NSEG):
                    nc.tensor.matmul(ps[:], lhsT=hT_sb[:, s, :], rhs=aT_sb[m][:, s, :],
                                     start=(s == 0), stop=(s == NSEG - 1))
                pre = work_pool.tile([B, H], fp16, tag="qkvpre")
                nc.vector.tensor_add(pre[:], ps[:], bb_sb[m][:])
                t = const_pool.tile([B, H], f32, tag=f"qkv{m}")
                nc.scalar.activation(t[:], pre[:], SILU, bias=0.0, scale=1.0)
                qkv_sb[m] = t
                if m == 1:  # K -> ktil -> ktil8 roundtrip
                    kmax = work_pool.tile([B, 1], f32, tag="kmax")
                    nc.vector.tensor_reduce(kmax[:], t[:], axis=AX, op=MAX)
                    ktil = const_pool.tile([B, H], fp16)
                    nc.vector.tensor_scalar_sub(ktil[:], t[:], kmax[:])
                    nc.sync.dma_start(ktil_dram[:], ktil[:])
                    # stationary blocks for the S MMs: [(r,s), pair, j]
                    ktil8_sb = const_pool.tile([8, 32, 128], fp16)
                    ksrc = ktil_dram[:].rearrange(
                        "(pp r) (s j) -> (r s) pp j", r=2, s=NSEG)
                    nc.sync.dma_start(ktil8_sb[:], ksrc)
                elif m == 0:  # Q -> q16 -> broadcast
                    qfp = work_pool.tile([B, H], fp16, tag="qfp")
                    nc.vector.tensor_copy(qfp[:], t[:])
                    nc.sync.dma_start(q_dram[:], qfp[:])
                    # q broadcasts: gpsimd SWDGE only (sync HWDGE takes
                    # 9-27us to generate a broadcast pattern and serializes
                    # the queue); bottom DMAs also fill dead rows 47-63
                    qsrc = q_dram[:].rearrange("(gq x) i -> gq x i", x=4)
                    for half in range(2):
                        g0 = 8 * half
                        src = qsrc[g0:g0 + 8]
                        nc.gpsimd.dma_start(
                            qb_all[0:NK, g0:g0 + 8, :, :],
                            src[:, 0:2, :].unsqueeze(0).broadcast_to((NK, 8, 2, H)))
                        nc.gpsimd.dma_start(
                            qb_all[NK:NROW, g0:g0 + 8, :, :],
                            src[:, 2:4, :].unsqueeze(0).broadcast_to((NROW - NK, 8, 2, H)))
            q_sb, k_sb, v_sb = qkv_sb

            # vo_all[jj, g, s, (q,r,c)] = V[4g+2q+r, 128s+jj] (c=0) | 1 (c=1)
            vo_all = const_pool.tile([128, NG, NSEG, 8], fp16)
            nc.vector.memset(vo_all[:], 1.0)
            vT_ps = []
            for s in range(NSEG):
                pt = psum_mm.tile([128, B], f32, tag="tp")
                nc.tensor.transpose(pt[:], v_sb[:, 128 * s:128 * (s + 1)], eye_sb[:])
                vT_ps.append(pt)
                for qr in range(4):
                    nc.vector.tensor_copy(vo_all[:, :, s, 2 * qr], pt[:, qr::4])

        # late-needed params: emitted after phase 0 so the q-path DMAs
        # reach the head of the sync queue sooner
        tcol_sb = const_pool.tile([NROW, 1], f32)
        nc.sync.dma_start(tcol_sb[:], tcol_d[:])
        ntcol_sb = const_pool.tile([NROW, 1], f32)
        nc.sync.dma_start(ntcol_sb[:], ntcol_d[:])
        invdt_sb = const_pool.tile([B, NK], f32)
        nc.sync.dma_start(invdt_sb[:], invdt_d[:])
        w_outT_sb = const_pool.tile([128, NSEG, O], fp16)
        nc.sync.dma_start(w_outT_sb[:], w_outT_d[:].rearrange("(s p) o -> p s o", p=128))
        b_out_sb = const_pool.tile([B, O], f32)
        nc.sync.dma_start(b_out_sb[:], b_out_d[:])

        # =================== phase A ===================
        nden0 = const_pool.tile([32, 2, NP], f32, tag="nden0")
        nden1 = const_pool.tile([32, 2, NP], f32, tag="nden1")
        nden_h = [nden0, nden1]
        stage_nd = big_pool.tile([8, NG, 2, 2, NP], f32, tag="stagend")
        SFREE = 2 * NSEG * NP  # 384

        def drain_half(h):
            # nd garbage-cell filter: per (q, r) extract its diag col block
            for q in range(2):
                for r in range(2):
                    row = 4 * q + 2 * r
                    nc.sync.dma_start(
                        nden_h[h][2 * q + r:32:4, 0, :],
                        stage_nd[row:row + 1, 8 * h:8 * h + 8, q, r, :])
                    nc.sync.dma_start(
                        nden_h[h][2 * q + r:32:4, 1, :],
                        stage_nd[row + 1:row + 2, 8 * h:8 * h + 8, q, r, :])

        fn_half = [None, None]

        def node_math(h, psum_tp):
            dinv = work_pool.tile([32, NP], f32, tag="dinv")
            nc.vector.reciprocal(dinv[:], nden_h[h][:, 1, :])
            rn = work_pool.tile([32, NP], f32, tag="rn")
            nc.vector.tensor_mul(rn[:], nden_h[h][:, 0, :], dinv[:])
            fn = const_pool.tile([32, NP], f32, tag=f"fn{h}")
            nc.scalar.activation(fn[:], rn[:], SILU, bias=0.0, scale=1.0)
            fn_half[h] = fn
            slop = work_pool.tile([32, NK], f32, tag="slop")
            nc.vector.tensor_sub(slop[:], fn[:, 1:NP], fn[:, 0:NK])
            nc.vector.tensor_mul(slop[:], slop[:], invdt_sb[0:32, :])
            w_sb = work_pool.tile([32, NK], f32, tag="wsb")
            nc.vector.tensor_copy(w_sb[:, 0:1], slop[:, 0:1])
            nc.vector.tensor_sub(w_sb[:, 1:NK], slop[:, 1:NK], slop[:, 0:NK - 1])
            # transpose w twice: a PE transpose at base 0 for the top rows,
            # and a regular matmul (out = w_sb.T @ eye) landing at PSUM
            # partition 64 so the bottom scatter is a base-matched DVE copy
            pt_a = psum_tp.tile([128, B], f32, tag="tpwa")
            nc.tensor.transpose(pt_a[0:NK, 0:32], w_sb[:], eye_sb[0:32, 0:32])
            pt_b = psum_tp.tile([128, B], f32, tag="tpwb")
            nc.tensor.matmul(pt_b[BOT:NROW, 0:32], lhsT=w_sb[:],
                             rhs=eye_sb[0:32, 0:32], start=True, stop=True)
            # scatter: sample b=32h+m has its w col at flat 1024h+33m — top
            # knot-rows if pair(b) even (m%4<2) else the base-64 bottom rows
            for mm in range(4):
                c0 = 1024 * h + 33 * mm
                if mm < 2:
                    nc.vector.tensor_copy(
                        zbig[0:NK, c0:c0 + 33 * 28 + 1:33 * 4],
                        pt_a[0:NK, mm:32:4])
                else:
                    nc.vector.tensor_copy(
                        zbig[BOT:NROW, c0:c0 + 33 * 28 + 1:33 * 4],
                        pt_b[BOT:NROW, mm:32:4])

        with tc.tile_pool(name="pss", bufs=2, space="PSUM") as psum_s, \
             tc.tile_pool(name="psnd", bufs=2, space="PSUM") as psum_nd, \
             tc.tile_pool(name="pstp", bufs=1, space="PSUM") as psum_tp:
            for g in range(NG):
                s_ps = psum_s.tile([128, 2, 512], f32, tag="s")
                for q in range(2):
                    pp = 2 * g + q
                    nc.tensor.matmul(
                        s_ps[:, q, 0:SFREE],
                        lhsT=ktil8_sb[:, pp, :],
                        rhs=qhat2_sb[:],
                        start=True, stop=True)
                e_t = e_pool.tile([128, 2, 2, NSEG, NP], bf16, tag="e")
                nc.scalar.activation(
                    e_t[:].rearrange("j q r s n -> j q (r s n)"),
                    s_ps[:, :, 0:SFREE], EXP, bias=0.0, scale=1.0)
                nd_ps = psum_nd.tile([8, 2, 2, NP], f32, tag="nd")
                for s in range(NSEG):
                    nc.tensor.matmul(
                        nd_ps[:],
                        lhsT=vo_all[:, g, s, :],
                        rhs=e_t[:, :, :, s, :],
                        start=(s == 0), stop=(s == NSEG - 1))
                nc.vector.tensor_copy(stage_nd[:, g], nd_ps[:])
                if g == 7:
                    drain_half(0)
            drain_half(1)
            node_math(0, psum_tp)

        # =================== phase B ===================
        ctx_sb = const_pool.tile([B, H], f32, tag="ctx")
        with tc.tile_pool(name="pscx", bufs=2, space="PSUM") as psum_cx, \
             tc.tile_pool(name="pstpB", bufs=1, space="PSUM") as psum_tpB:
            ctx_ps = None
            for g in range(NG):
                if g == 2:
                    # second-half node math: needed from sample 32 (g=8);
                    # emitted here so its engine ops overlap early phase B
                    node_math(1, psum_tpB)
                bas = bas_pool.tile([NROW, 2, H], fp16, tag="bas")
                if g % 3 == 2:
                    nc.scalar.activation(bas[:], qb_all[:, g, :, :], RELU,
                                         bias=ntcol_sb[:], scale=1.0)
                else:
                    nc.vector.tensor_scalar(
                        bas[:], qb_all[:, g, :, :], tcol_sb[:], tcol_sb[:],
                        op0=MAX, op1=SUB)
                for bi in range(4):
                    b = 4 * g + bi
                    r = b % 2
                    if b % 32 == 0:
                        ctx_ps = psum_cx.tile([32, H], f32, tag="ctxps")
                    nc.tensor.matmul(
                        ctx_ps[:],
                        lhsT=zbig[:, 32 * b:32 * (b + 1)],
                        rhs=bas[:, r, :],
                        start=(b % 32 == 0), stop=(b % 32 == 31))
                    if b == 31:
                        nc.vector.tensor_scalar_add(
                            ctx_sb[0:32, :], ctx_ps[:], fn_half[0][:, 0:1])
                    elif b == 63:
                        cst = stage_pool.tile([32, H], f32, tag="cxst")
                        nc.vector.tensor_scalar_add(cst[:], ctx_ps[:], fn_half[1][:, 0:1])
                        nc.sync.dma_start(ctx_sb[32:64, :], cst[:])

            # ctx_T (F(t0) already added in the drains)
            ctxT_sb = work_pool.tile([128, NSEG, B], fp16, tag="ctxT")
            for s in range(NSEG):
                transpose_to(psum_tpB, ctx_sb[:, 128 * s:128 * (s + 1)], ctxT_sb[:, s, :])

            # y = silu(ctx @ W_out.T + b_out)
            y_ps = psum_tpB.tile([B, O], f32, tag="y")
            for s in range(NSEG):
                nc.tensor.matmul(y_ps[:], lhsT=ctxT_sb[:, s, :], rhs=w_outT_sb[:, s, :],
                                 start=(s == 0), stop=(s == NSEG - 1))
            y_pre = work_pool.tile([B, O], f32, tag="ypre")
            nc.vector.tensor_add(y_pre[:], y_ps[:], b_out_sb[:])
        y_sb = work_pool.tile([B, O], f32, tag="y")
        nc.scalar.activation(y_sb[:], y_pre[:], SILU, bias=0.0, scale=1.0)

        # ---- tail: block-diag quadratic form ----
        y2 = work_pool.tile([B, O], f32, tag="y2")
        nc.vector.tensor_mul(y2[:], y_sb[:], y_sb[:])
        m_sb = work_pool.tile([B, 5], f32, tag="m5")
        nc.vector.tensor_reduce(
            m_sb[:], y2[:].rearrange("p (a b) -> p a b", b=5), axis=AX,
            op=mybir.AluOpType.add)
        p2 = work_pool.tile([B, 2], f32, tag="p2")
        nc.vector.tensor_add(p2[:], y2[:, 0:3:2], y2[:, 1:4:2])
        c2 = work_pool.tile([B, 2], f32, tag="c2")
        nc.vector.tensor_mul(c2[:], y_sb[:, 0:2], y_sb[:, 2:4])
        cc = work_pool.tile([B, 1], f32, tag="cc")
        nc.vector.tensor_add(cc[:], c2[:, 0:1], c2[:, 1:2])
        mm12 = work_pool.tile([B, 1], f32, tag="mm12")
        nc.vector.tensor_add(mm12[:], m_sb[:, 1:2], m_sb[:, 2:3])
        mp = work_pool.tile([B, 2], f32, tag="mp")
        nc.vector.tensor_mul(mp[:], m_sb[:, 0:4:3], p2[:])
        acc = work_pool.tile([B, 1], f32, tag="acc")
        nc.vector.tensor_add(acc[:], mp[:, 0:1], mp[:, 1:2])
        acc2 = work_pool.tile([B, 1], f32, tag="acc2")
        nc.vector.tensor_mul(acc2[:], mm12[:], cc[:])
        acc3 = work_pool.tile([B, 1], f32, tag="acc3")
        nc.vector.tensor_add(acc3[:], acc[:], acc2[:])
        res = work_pool.tile([B, 1], f32, tag="res")
        nc.vector.tensor_add(res[:], acc3[:], m_sb[:, 4:5])
        nc.sync.dma_start(out_d[:], res[:])

    nc.finalize()
    return nc


def _host_inputs(x, W_in, b_in, Aq, Bq, Ak, Bk, Av, Bv, W_out, b_out):
    """Build the per-core input maps (shard x over batch; params replicated)."""
    f = lambda a: np.ascontiguousarray(a, dtype=np.float32)
    h16 = lambda a: np.ascontiguousarray(a, dtype=np.float16)
    t = _grid()  # already fp16-rounded values in f32
    # qhat2[(r,s), (r',s',p)] = t_p if r==r' and s==s' else 0
    qhat2 = np.zeros((8, 2 * NSEG * NP), np.float16)
    for r in range(2):
        for s in range(NSEG):
            col0 = (r * NSEG + s) * NP
            qhat2[r * NSEG + s, col0:col0 + NP] = t.astype(np.float16)
    tk = t[:NK]
    # rows 0-46: knots (top pair); 47-63: 1e4 filler (basis exactly 0);
    # 64-110: knots (bottom pair)
    tcol111 = np.full((BOT + NK, 1), 1e4, np.float32)
    tcol111[:NK, 0] = tk
    tcol111[BOT:, 0] = tk
    ntcol111 = -tcol111
    dt_ = t[1:] - t[:-1]
    common = {
        "w_inT": h16(W_in.T),
        "aqT": h16(Aq.T),
        "akT": h16(Ak.T),
        "avT": h16(Av.T),
        "b_in_bc": h16(np.broadcast_to(b_in, (B, H))),
        "bq_bc": h16(np.broadcast_to(Bq, (B, H))),
        "bk_bc": h16(np.broadcast_to(Bk, (B, H))),
        "bv_bc": h16(np.broadcast_to(Bv, (B, H))),
        "w_outT": h16(W_out.T),
        "b_out_bc": f(np.broadcast_to(b_out, (B, O))),
        "eye64": f(np.eye(B)),
        "qhat2": qhat2,
        "tcol111": tcol111,
        "ntcol111": ntcol111,
        "invdt_bc": f(np.broadcast_to(1.0 / dt_[:NK], (B, NK))),
    }
    xs = f(x).reshape(N_CORES, B, IN)
    return [dict(common, xT=h16(xs[i].T)) for i in range(N_CORES)]


def _get_nc():
    if "nc" not in _cache:
        _cache["nc"] = _build_nc()
    return _cache["nc"]


def run_spmd(in_maps, trace=False):
    from concourse.bass_utils import run_bass_kernel_spmd

    nc = _get_nc()
    res = run_bass_kernel_spmd(nc, in_maps, core_ids=list(range(N_CORES)), trace=trace)
    return res


def kernel(x, na=None, W_in=None, b_in=None, Aq=None, Bq=None, Ak=None, Bk=None,
           Av=None, Bv=None, W_out=None, b_out=None):
    in_maps = _host_inputs(x, W_in, b_in, Aq, Bq, Ak, Bk, Av, Bv, W_out, b_out)
    res = run_spmd(in_maps)
    out = np.concatenate([r["out"] for r in res.results], axis=0)
    return out.astype(np.float32)
